# revision 3
# baseline (speedup 1.0000x reference)
"""FP8-palettized linear kernel for 8x TRN2 NeuronCores.

Computes: out[b,s,o] = sum_d input[b,s,d] * lookup_table[weight[o,d]] + bias[o]
with input [4,2048,4096] f32, weight [4096,4096] int32 (palette ids < 256),
lookup_table [256] f32, bias [4096] f32.

Strategy (column-parallel, per sharding hint):
  - Each core owns a 512-wide slice of out_features; input replicated.
  - Dequantization runs on the ScalarE (ACT) spline evaluator: at
    kernel() time we bake the 256-entry palette into a custom PWP
    activation table (a piecewise-constant staircase f(x) = LUT[round(x)]
    hijacking the 'gelu' slot, handed to walrus via
    BASS_ACT_ROOT_JSON_PATH — see act_table.py). Palette indices are
    shipped as bf16 (exact for 0..255); one ACTIVATE per k-tile turns
    the [128, 512] index tile into the bf16 W^T tile at 1 elem/cycle/lane
    (~720ns/tile), bit-identical to gather+round dequant.
  - Host prep is layout/dtype marshalling only: X tiled into contiguous
    [128, 4096] X^T slabs (one 1MB DMA per m-tile), weight indices
    transposed into k-tile-major [128, osh] tiles as bf16.
  - TensorE accumulates X^T-slab @ W^T over 32 k-tiles in PSUM; the first
    8 m-tiles run k-outer across the 8 PSUM banks so the PE starts while
    dequant + X DMA are still streaming; remaining m-tiles run m-outer
    with rotating banks. DVE adds bias, results DMA out per m-tile.
  - The LUT content is folded into the widx tensor name so the NEFF cache
    key changes whenever the activation table must change.
"""

import hashlib
import os
import tempfile

import ml_dtypes
import numpy as np

import concourse.bacc as bacc
import concourse.mybir as mybir
import concourse.tile as tile
from concourse.bass_utils import run_bass_kernel_spmd

import act_table

P = 128
N_CORES = 8

# Full-problem dims (hardcoded per harness contract).
BATCH, SEQ, D_IN, D_OUT, PALETTE = 4, 2048, 4096, 4096, 256
M_FULL = BATCH * SEQ  # 8192

MM_DTYPE = mybir.dt.bfloat16


def _np_mm_dtype():
    return ml_dtypes.bfloat16


def lut_tag(lookup_table):
    lut = np.asarray(lookup_table, dtype=np.float32).reshape(PALETTE)
    return hashlib.sha1(lut.tobytes()).hexdigest()[:12]


def install_act_tables(lookup_table):
    """Bake the palette into a custom ACT table root and point walrus at
    it. Must run before the NEFF compile."""
    lut = np.asarray(lookup_table, dtype=np.float32).reshape(PALETTE)
    root = act_table.build_act_root(
        lut, tempfile.mkdtemp(prefix=f"actroot_{lut_tag(lut)}_"))
    os.environ["BASS_ACT_ROOT_JSON_PATH"] = root
    return root


def build_program(nc, *, m, k, osh, ltag, reps=1):
    """Emit the per-core Tile program. m: rows of X (mult of 128), k: d dim
    (mult of 128), osh: out-features per core (512). reps>1 wraps the body
    in a hardware loop (benchmarking: amortizes dispatch overhead).
    ltag: hash of the lookup table (cache-keys the NEFF to the act root)."""
    n_kt = k // P
    n_mt = m // P
    np1 = 8  # phase-1 m-tiles, one per PSUM bank

    xt = nc.dram_tensor("xt", [m, k], MM_DTYPE, kind="ExternalInput")
    widx = nc.dram_tensor(f"widx_{ltag}", [n_kt * P, osh], MM_DTYPE,
                          kind="ExternalInput")
    bias = nc.dram_tensor("bias", [P, osh], mybir.dt.float32,
                          kind="ExternalInput")
    out = nc.dram_tensor("out", [m, osh], mybir.dt.float32,
                         kind="ExternalOutput")

    with tile.TileContext(nc) as tc:
        with (
            tc.tile_pool(name="const", bufs=1) as const_pool,
            tc.tile_pool(name="idx", bufs=2) as idx_pool,
            tc.tile_pool(name="wt", bufs=2) as wt_pool,
            tc.tile_pool(name="xs", bufs=2) as x_pool,
            tc.tile_pool(name="psum", bufs=1, space="PSUM") as psum_pool,
            tc.tile_pool(name="osb", bufs=3) as osb_pool,
        ):
            bias_sb = const_pool.tile([P, osh], mybir.dt.float32, tag="bsb")
            nc.sync.dma_start(bias_sb[:], bias[:])

            def rep_body():
                # --- dequant: ACT staircase turns idx tiles into W^T ---
                wt_tiles = []
                for kt in range(n_kt):
                    idxt = idx_pool.tile([P, osh], MM_DTYPE,
                                         tag="idx", name=f"idx{kt}")
                    nc.scalar.dma_start(idxt[:],
                                        widx[kt * P:(kt + 1) * P, :])
                    wt = wt_pool.tile([P, osh], MM_DTYPE,
                                      tag=f"wt{kt:02d}", name=f"wt{kt}")
                    nc.scalar.activation(
                        wt[:], idxt[:], mybir.ActivationFunctionType.Gelu)
                    wt_tiles.append(wt)

                def load_xslab(mt):
                    xslab = x_pool.tile([P, k], MM_DTYPE,
                                        tag=f"xs{mt % np1}", name=f"xs{mt}")
                    nc.sync.dma_start(xslab[:], xt[mt * P:(mt + 1) * P, :])
                    return xslab

                def finish_mtile(mt, psum):
                    osb = osb_pool.tile([P, osh], mybir.dt.float32,
                                        tag="osb", name=f"osb{mt}")
                    nc.vector.tensor_tensor(
                        osb[:], psum[:], bias_sb[:], op=mybir.AluOpType.add)
                    nc.scalar.dma_start(out[mt * P:(mt + 1) * P, :], osb[:])

                phase1 = int(os.environ.get("PAL_PHASE1", "1"))
                if phase1:
                    # --- phase 1: np1 m-tiles k-outer across PSUM banks ---
                    slabs = [load_xslab(mt) for mt in range(np1)]
                    psums = [psum_pool.tile([P, osh], mybir.dt.float32,
                                            tag=f"ps{i}", name=f"ps{i}")
                             for i in range(np1)]
                    for kt in range(n_kt):
                        for i in range(np1):
                            nc.tensor.matmul(
                                psums[i][:],
                                lhsT=slabs[i][:, kt * P:(kt + 1) * P],
                                rhs=wt_tiles[kt][:],
                                start=(kt == 0),
                                stop=(kt == n_kt - 1))
                    for i in range(np1):
                        finish_mtile(i, psums[i])

                # --- phase 2: remaining m-tiles, m-outer ---
                for mt in range(np1 if phase1 else 0, n_mt):
                    xslab = load_xslab(mt)
                    psum = psum_pool.tile([P, osh], mybir.dt.float32,
                                          tag=f"ps{mt % np1}",
                                          name=f"ps{mt}")
                    for kt in range(n_kt):
                        nc.tensor.matmul(
                            psum[:],
                            lhsT=xslab[:, kt * P:(kt + 1) * P],
                            rhs=wt_tiles[kt][:],
                            start=(kt == 0),
                            stop=(kt == n_kt - 1))
                    finish_mtile(mt, psum)

            if reps > 1:
                with tc.For_i(0, reps, 1):
                    rep_body()
            else:
                rep_body()

    return xt, widx, bias, out


def make_core_inputs(input, lookup_table, weight, bias, *, m=M_FULL, k=D_IN,
                     osh=D_OUT // N_CORES, n_cores=N_CORES):
    """Host-side sharding/layout prep (no palette lookups). Returns in_maps.
    Also installs the act-table root for the palette (env for the compile)."""
    install_act_tables(lookup_table)
    ltag = lut_tag(lookup_table)

    n_kt = k // P
    n_mt = m // P
    x2 = np.asarray(input, dtype=np.float32).reshape(m, k)
    # xt[mt, p, kt*128+j] = X[mt*128+j, kt*128+p]
    xt = (x2.reshape(n_mt, P, n_kt, P).transpose(0, 3, 2, 1)
          .reshape(m, k).astype(_np_mm_dtype()))

    weight = np.asarray(weight)
    bias = np.asarray(bias, dtype=np.float32)

    in_maps = []
    for c in range(n_cores):
        w_shard = weight[c * osh:(c + 1) * osh, :]  # [osh, k] int32
        # widx[kt*128+p, o] = weight[c*osh+o, kt*128+p] as bf16 (ids < 256
        # are exact in bf16)
        widx = w_shard.T.astype(_np_mm_dtype())
        in_maps.append({
            "xt": xt,
            f"widx_{ltag}": np.ascontiguousarray(widx),
            "bias": np.broadcast_to(bias[c * osh:(c + 1) * osh],
                                    (P, osh)).copy(),
        })
    return in_maps


def kernel(input, lookup_table, weight, bias, *, trace=False):
    osh = D_OUT // N_CORES
    in_maps = make_core_inputs(input, lookup_table, weight, bias)

    nc = bacc.Bacc("TRN2", target_bir_lowering=False, debug=False,
                   num_devices=N_CORES)
    build_program(nc, m=M_FULL, k=D_IN, osh=osh, ltag=lut_tag(lookup_table))
    nc.compile()

    res = run_bass_kernel_spmd(nc, in_maps, core_ids=list(range(N_CORES)),
                               trace=trace)
    out = np.concatenate([r["out"] for r in res.results], axis=1)
    out = np.ascontiguousarray(out.reshape(BATCH, SEQ, D_OUT),
                               dtype=np.float32)
    if trace:
        kernel.last_results = res
    return out


# revision 7
# speedup vs baseline: 7.3907x; 7.3907x over previous
"""FP8-palettized linear kernel for 8x TRN2 NeuronCores.

Computes: out[b,s,o] = sum_d input[b,s,d] * lookup_table[weight[o,d]] + bias[o]
with input [4,2048,4096] f32, weight [4096,4096] int32 (palette ids < 256),
lookup_table [256] f32, bias [4096] f32.

Strategy (column-parallel, per sharding hint):
  - Each core owns a 512-wide slice of out_features; input replicated.
  - Dequantization runs on the ScalarE (ACT) spline evaluator: at
    kernel() time we bake the 256-entry palette into a custom PWP
    activation table (a piecewise-constant staircase f(x) = LUT[round(x)]
    hijacking the 'gelu' slot, handed to walrus via
    BASS_ACT_ROOT_JSON_PATH — see act_table.py). Palette indices are
    shipped as bf16 (exact for 0..255); one ACTIVATE per k-tile turns
    the [128, 512] index tile into the bf16 W^T tile at 1 elem/cycle/lane
    (~720ns/tile), bit-identical to gather+round dequant.
  - Host prep is layout/dtype marshalling only: X tiled into contiguous
    [128, 4096] X^T slabs (one 1MB DMA per m-tile), weight indices
    transposed into k-tile-major [128, osh] tiles as bf16.
  - TensorE accumulates X^T-slab @ W^T over 32 k-tiles in PSUM; the first
    8 m-tiles run k-outer across the 8 PSUM banks so the PE starts while
    dequant + X DMA are still streaming; remaining m-tiles run m-outer
    with rotating banks. DVE adds bias, results DMA out per m-tile.
  - The LUT content is folded into the widx tensor name so the NEFF cache
    key changes whenever the activation table must change.
"""

import hashlib
import os
import tempfile

import ml_dtypes
import numpy as np

import concourse.bacc as bacc
import concourse.mybir as mybir
import concourse.tile as tile
from concourse.bass_utils import run_bass_kernel_spmd

import act_table

P = 128
N_CORES = 8

# Full-problem dims (hardcoded per harness contract).
BATCH, SEQ, D_IN, D_OUT, PALETTE = 4, 2048, 4096, 4096, 256
M_FULL = BATCH * SEQ  # 8192

MM_DTYPE = mybir.dt.bfloat16


def _np_mm_dtype():
    return ml_dtypes.bfloat16


def lut_tag(lookup_table):
    lut = np.asarray(lookup_table, dtype=np.float32).reshape(PALETTE)
    return hashlib.sha1(lut.tobytes()).hexdigest()[:12]


def install_act_tables(lookup_table):
    """Bake the palette into a custom ACT table root and point walrus at
    it. Must run before the NEFF compile."""
    lut = np.asarray(lookup_table, dtype=np.float32).reshape(PALETTE)
    root = act_table.build_act_root(
        lut, tempfile.mkdtemp(prefix=f"actroot_{lut_tag(lut)}_"))
    os.environ["BASS_ACT_ROOT_JSON_PATH"] = root
    return root


def build_program(nc, *, m, k, osh, ltag, reps=1):
    """Emit the per-core Tile program. m: rows of X (mult of 128), k: d dim
    (mult of 128), osh: out-features per core (512). reps>1 wraps the body
    in a hardware loop (benchmarking: amortizes dispatch overhead).
    ltag: hash of the lookup table (cache-keys the NEFF to the act root)."""
    n_kt = k // P
    n_mt = m // P
    n_mt = int(os.environ.get("PAL_NMT", str(n_mt)))  # timing bisect only
    np1 = int(os.environ.get("PAL_NP1", "8"))  # m-tiles per k-outer group

    xt = nc.dram_tensor("xt", [m, k], MM_DTYPE, kind="ExternalInput")
    widx = nc.dram_tensor(f"widx_{ltag}", [n_kt * P, osh], MM_DTYPE,
                          kind="ExternalInput")
    bias = nc.dram_tensor("bias", [P, osh], mybir.dt.float32,
                          kind="ExternalInput")
    out = nc.dram_tensor("out", [m, osh], mybir.dt.float32,
                         kind="ExternalOutput")

    with tile.TileContext(nc) as tc:
        with (
            tc.tile_pool(name="const", bufs=1) as const_pool,
            tc.tile_pool(name="idx", bufs=2) as idx_pool,
            tc.tile_pool(name="wt", bufs=2) as wt_pool,
            tc.tile_pool(name="xs", bufs=2) as x_pool,
            tc.tile_pool(name="psum", bufs=1, space="PSUM") as psum_pool,
            tc.tile_pool(name="osb", bufs=3) as osb_pool,
        ):
            bias_sb = const_pool.tile([P, osh], mybir.dt.float32, tag="bsb")
            nc.sync.dma_start(bias_sb[:], bias[:])

            def rep_body():
                # --- dequant: ACT staircase turns idx tiles into W^T ---
                wt_tiles = []
                for kt in range(n_kt):
                    idxt = idx_pool.tile([P, osh], MM_DTYPE,
                                         tag="idx", name=f"idx{kt}")
                    nc.scalar.dma_start(idxt[:],
                                        widx[kt * P:(kt + 1) * P, :])
                    wt = wt_pool.tile([P, osh], MM_DTYPE,
                                      tag=f"wt{kt:02d}", name=f"wt{kt}")
                    nc.scalar.activation(
                        wt[:], idxt[:], mybir.ActivationFunctionType.Gelu)
                    wt_tiles.append(wt)

                def load_xslab(mt):
                    xslab = x_pool.tile([P, k], MM_DTYPE,
                                        tag=f"xs{mt % np1}", name=f"xs{mt}")
                    nc.sync.dma_start(xslab[:], xt[mt * P:(mt + 1) * P, :])
                    return xslab

                def finish_mtile(mt, psum):
                    osb = osb_pool.tile([P, osh], mybir.dt.float32,
                                        tag="osb", name=f"osb{mt}")
                    nc.vector.tensor_tensor(
                        osb[:], psum[:], bias_sb[:], op=mybir.AluOpType.add)
                    nc.scalar.dma_start(out[mt * P:(mt + 1) * P, :], osb[:])

                # --- matmul: groups of np1 m-tiles, k-outer across PSUM
                # banks within each group (bank-interleaved accumulation;
                # long single-bank chains measure ~10x slower on HW) ---
                for g in range(n_mt // np1):
                    slabs = [load_xslab(g * np1 + i) for i in range(np1)]
                    psums = [psum_pool.tile(
                        [P, osh], mybir.dt.float32,
                        tag=f"ps{(g % (8 // np1)) * np1 + i}",
                        name=f"ps{g}_{i}") for i in range(np1)]
                    for kt in range(n_kt):
                        for i in range(np1):
                            nc.tensor.matmul(
                                psums[i][:],
                                lhsT=slabs[i][:, kt * P:(kt + 1) * P],
                                rhs=wt_tiles[kt][:],
                                start=(kt == 0),
                                stop=(kt == n_kt - 1))
                    for i in range(np1):
                        finish_mtile(g * np1 + i, psums[i])

            if reps > 1:
                with tc.For_i(0, reps, 1):
                    rep_body()
            else:
                rep_body()

    return xt, widx, bias, out


def make_core_inputs(input, lookup_table, weight, bias, *, m=M_FULL, k=D_IN,
                     osh=D_OUT // N_CORES, n_cores=N_CORES):
    """Host-side sharding/layout prep (no palette lookups). Returns in_maps.
    Also installs the act-table root for the palette (env for the compile)."""
    install_act_tables(lookup_table)
    ltag = lut_tag(lookup_table)

    n_kt = k // P
    n_mt = m // P
    x2 = np.asarray(input, dtype=np.float32).reshape(m, k)
    # xt[mt, p, kt*128+j] = X[mt*128+j, kt*128+p]
    xt = (x2.reshape(n_mt, P, n_kt, P).transpose(0, 3, 2, 1)
          .reshape(m, k).astype(_np_mm_dtype()))

    weight = np.asarray(weight)
    bias = np.asarray(bias, dtype=np.float32)

    in_maps = []
    for c in range(n_cores):
        w_shard = weight[c * osh:(c + 1) * osh, :]  # [osh, k] int32
        # widx[kt*128+p, o] = weight[c*osh+o, kt*128+p] as bf16 (ids < 256
        # are exact in bf16)
        widx = w_shard.T.astype(_np_mm_dtype())
        in_maps.append({
            "xt": xt,
            f"widx_{ltag}": np.ascontiguousarray(widx),
            "bias": np.broadcast_to(bias[c * osh:(c + 1) * osh],
                                    (P, osh)).copy(),
        })
    return in_maps


def kernel(input, lookup_table, weight, bias, *, trace=False):
    osh = D_OUT // N_CORES
    in_maps = make_core_inputs(input, lookup_table, weight, bias)

    nc = bacc.Bacc("TRN2", target_bir_lowering=False, debug=False,
                   num_devices=N_CORES)
    build_program(nc, m=M_FULL, k=D_IN, osh=osh, ltag=lut_tag(lookup_table))
    nc.compile()

    res = run_bass_kernel_spmd(nc, in_maps, core_ids=list(range(N_CORES)),
                               trace=trace)
    out = np.concatenate([r["out"] for r in res.results], axis=1)
    out = np.ascontiguousarray(out.reshape(BATCH, SEQ, D_OUT),
                               dtype=np.float32)
    if trace:
        kernel.last_results = res
    return out


# revision 15
# speedup vs baseline: 8.2758x; 1.1198x over previous
"""FP8-palettized linear kernel for 8x TRN2 NeuronCores.

Computes: out[b,s,o] = sum_d input[b,s,d] * lookup_table[weight[o,d]] + bias[o]
with input [4,2048,4096] f32, weight [4096,4096] int32 (palette ids < 256),
lookup_table [256] f32, bias [4096] f32.

Strategy (column-parallel, per sharding hint):
  - Each core owns a 512-wide slice of out_features; input replicated.
  - Dequantization runs on the ScalarE (ACT) spline evaluator: at
    kernel() time we bake the 256-entry palette into a custom PWP
    activation table (a piecewise-constant staircase f(x) = LUT[round(x)]
    hijacking the 'gelu' slot, handed to walrus via
    BASS_ACT_ROOT_JSON_PATH — see act_table.py). Palette indices are
    shipped as bf16 (exact for 0..255); one ACTIVATE per k-tile turns
    the [128, 512] index tile into the bf16 W^T tile at 1 elem/cycle/lane
    (~720ns/tile), bit-identical to gather+round dequant.
  - Host prep is layout/dtype marshalling only: X tiled into contiguous
    [128, 4096] X^T slabs (one 1MB DMA per m-tile), weight indices
    transposed into k-tile-major [128, osh] tiles as bf16.
  - TensorE accumulates X^T-slab @ W^T over 32 k-tiles in PSUM; the first
    8 m-tiles run k-outer across the 8 PSUM banks so the PE starts while
    dequant + X DMA are still streaming; remaining m-tiles run m-outer
    with rotating banks. DVE adds bias, results DMA out per m-tile.
  - The LUT content is folded into the widx tensor name so the NEFF cache
    key changes whenever the activation table must change.
"""

import hashlib
import os
import tempfile

import ml_dtypes
import numpy as np

import concourse.bacc as bacc
import concourse.mybir as mybir
import concourse.tile as tile
from concourse.bass_utils import run_bass_kernel_spmd

import act_table

P = 128
N_CORES = 8

# Full-problem dims (hardcoded per harness contract).
BATCH, SEQ, D_IN, D_OUT, PALETTE = 4, 2048, 4096, 4096, 256
M_FULL = BATCH * SEQ  # 8192

MM_DTYPE = mybir.dt.bfloat16


def _np_mm_dtype():
    return ml_dtypes.bfloat16


def lut_tag(lookup_table):
    lut = np.asarray(lookup_table, dtype=np.float32).reshape(PALETTE)
    return hashlib.sha1(lut.tobytes()).hexdigest()[:12]


def install_act_tables(lookup_table):
    """Bake the palette into a custom ACT table root and point walrus at
    it. Must run before the NEFF compile."""
    lut = np.asarray(lookup_table, dtype=np.float32).reshape(PALETTE)
    root = act_table.build_act_root(
        lut, tempfile.mkdtemp(prefix=f"actroot_{lut_tag(lut)}_"))
    os.environ["BASS_ACT_ROOT_JSON_PATH"] = root
    return root


def build_program(nc, *, m, k, osh, ltag, reps=1):
    """Emit the per-core Tile program. m: rows of X (mult of 128), k: d dim
    (mult of 128), osh: out-features per core (512). reps>1 wraps the body
    in a hardware loop (benchmarking: amortizes dispatch overhead).
    ltag: hash of the lookup table (cache-keys the NEFF to the act root)."""
    n_kt = k // P
    n_mt = m // P
    n_mt = int(os.environ.get("PAL_NMT", str(n_mt)))  # timing bisect only
    np1 = int(os.environ.get("PAL_NP1", "8"))  # m-tiles per k-outer group

    xt = nc.dram_tensor("xt", [m, k], MM_DTYPE, kind="ExternalInput")
    # widx[p, kt*osh + o] = palette id of W^T[kt*128+p, o] (bf16-exact)
    widx = nc.dram_tensor(f"widx_{ltag}", [P, n_kt * osh], MM_DTYPE,
                          kind="ExternalInput")
    bias = nc.dram_tensor("bias", [P, osh], mybir.dt.float32,
                          kind="ExternalInput")
    out = nc.dram_tensor("out", [m, osh], mybir.dt.float32,
                         kind="ExternalOutput")

    with tile.TileContext(nc) as tc:
        with (
            tc.tile_pool(name="const", bufs=1) as const_pool,
            tc.tile_pool(name="idx", bufs=1) as idx_pool,
            tc.tile_pool(name="wt", bufs=2) as wt_pool,
            tc.tile_pool(name="xs", bufs=2) as x_pool,
            tc.tile_pool(name="psum", bufs=1, space="PSUM") as psum_pool,
            tc.tile_pool(name="osb", bufs=3) as osb_pool,
        ):
            bias_sb = const_pool.tile([P, osh], mybir.dt.float32, tag="bsb")
            nc.sync.dma_start(bias_sb[:], bias[:])

            act_split = int(os.environ.get("PAL_ACTSPLIT", "2"))
            kt_chunk = n_kt // act_split

            def rep_body():
                # --- dequant: batched ACT staircase turns idx into W^T;
                # few big ACTIVATEs amortize the act-table load ---
                wt_tiles = []
                for a in range(act_split):
                    idxt = idx_pool.tile([P, kt_chunk * osh], MM_DTYPE,
                                         tag=f"idx{a % 2}", name=f"idx{a}")
                    nc.scalar.dma_start(
                        idxt[:],
                        widx[:, a * kt_chunk * osh:(a + 1) * kt_chunk * osh])
                    wt = wt_pool.tile([P, kt_chunk * osh], MM_DTYPE,
                                      tag=f"wt{a}", name=f"wt{a}")
                    nc.scalar.activation(
                        wt[:], idxt[:], mybir.ActivationFunctionType.Gelu)
                    for j in range(kt_chunk):
                        wt_tiles.append(wt[:, j * osh:(j + 1) * osh])

                kh = k // 2  # half-slab columns (SBUF budget)

                def load_xslab(mt, half):
                    xslab = x_pool.tile([P, kh], MM_DTYPE,
                                        tag=f"xs{mt % np1}",
                                        name=f"xs{mt}_{half}")
                    nc.sync.dma_start(
                        xslab[:],
                        xt[mt * P:(mt + 1) * P, half * kh:(half + 1) * kh])
                    return xslab

                def finish_mtile(mt, psum):
                    osb = osb_pool.tile([P, osh], mybir.dt.float32,
                                        tag="osb", name=f"osb{mt}")
                    nc.vector.tensor_tensor(
                        osb[:], psum[:], bias_sb[:], op=mybir.AluOpType.add)
                    nc.scalar.dma_start(out[mt * P:(mt + 1) * P, :], osb[:])

                # --- matmul: groups of np1 m-tiles, k-outer across PSUM
                # banks within each group (bank-interleaved accumulation;
                # long single-bank chains measure ~10x slower on HW).
                # X slabs stream in k-halves to fit SBUF. ---
                nkh = n_kt // 2
                for g in range(n_mt // np1):
                    psums = [psum_pool.tile(
                        [P, osh], mybir.dt.float32,
                        tag=f"ps{(g % (8 // np1)) * np1 + i}",
                        name=f"ps{g}_{i}") for i in range(np1)]
                    for half in range(2):
                        slabs = [load_xslab(g * np1 + i, half)
                                 for i in range(np1)]
                        for j in range(nkh):
                            kt = half * nkh + j
                            for i in range(np1):
                                nc.tensor.matmul(
                                    psums[i][:],
                                    lhsT=slabs[i][:, j * P:(j + 1) * P],
                                    rhs=wt_tiles[kt][:],
                                    start=(kt == 0),
                                    stop=(kt == n_kt - 1))
                    for i in range(np1):
                        finish_mtile(g * np1 + i, psums[i])

            if reps > 1:
                with tc.For_i(0, reps, 1):
                    rep_body()
            else:
                rep_body()

    return xt, widx, bias, out


def make_core_inputs(input, lookup_table, weight, bias, *, m=M_FULL, k=D_IN,
                     osh=D_OUT // N_CORES, n_cores=N_CORES):
    """Host-side sharding/layout prep (no palette lookups). Returns in_maps.
    Also installs the act-table root for the palette (env for the compile)."""
    install_act_tables(lookup_table)
    ltag = lut_tag(lookup_table)

    n_kt = k // P
    n_mt = m // P
    x2 = np.asarray(input, dtype=np.float32).reshape(m, k)
    # xt[mt, p, kt*128+j] = X[mt*128+j, kt*128+p]
    xt = (x2.reshape(n_mt, P, n_kt, P).transpose(0, 3, 2, 1)
          .reshape(m, k).astype(_np_mm_dtype()))

    weight = np.asarray(weight)
    bias = np.asarray(bias, dtype=np.float32)

    in_maps = []
    for c in range(n_cores):
        w_shard = weight[c * osh:(c + 1) * osh, :]  # [osh, k] int32
        # widx[p, kt*osh+o] = weight[c*osh+o, kt*128+p] as bf16 (ids < 256
        # are exact in bf16)
        widx = (w_shard.T.reshape(n_kt, P, osh).transpose(1, 0, 2)
                .reshape(P, n_kt * osh).astype(_np_mm_dtype()))
        in_maps.append({
            "xt": xt,
            f"widx_{ltag}": np.ascontiguousarray(widx),
            "bias": np.broadcast_to(bias[c * osh:(c + 1) * osh],
                                    (P, osh)).copy(),
        })
    return in_maps


def kernel(input, lookup_table, weight, bias, *, trace=False):
    osh = D_OUT // N_CORES
    in_maps = make_core_inputs(input, lookup_table, weight, bias)

    nc = bacc.Bacc("TRN2", target_bir_lowering=False, debug=False,
                   num_devices=N_CORES)
    build_program(nc, m=M_FULL, k=D_IN, osh=osh, ltag=lut_tag(lookup_table))
    nc.compile()

    res = run_bass_kernel_spmd(nc, in_maps, core_ids=list(range(N_CORES)),
                               trace=trace)
    out = np.concatenate([r["out"] for r in res.results], axis=1)
    out = np.ascontiguousarray(out.reshape(BATCH, SEQ, D_OUT),
                               dtype=np.float32)
    if trace:
        kernel.last_results = res
    return out
